# revision 3
# baseline (speedup 1.0000x reference)
import sys
sys.path.insert(0, "/opt/trn_rl_repo")
import numpy as np

N = 100000
E = 800000
D = 128
P = 8
NSH = 12500          # nodes per core
NSH_PAD = 12544      # 98 * 128
ALPHA = 0.5
CHUNK = 32767        # real rows per gather chunk (slot 32767 of each chunk is zero)
NCHUNK = 4
CALL = 960           # idxs per gather/scatter call (ring-safe: <=61/121 descs)
CALL_PAD = 1024      # round_up(960,128)


def _plan_direction(gather_nodes, seg_nodes, core):
    """Build gather idx / scatter idx call lists for one direction on one core.

    gather_nodes: global node id whose x-row is gathered, per edge
    seg_nodes: shard-local [0,NSH) node receiving the row, per edge
    Returns list of (chunk_id, gidx_i16[960], sidx_i16[960]) calls.
    """
    chunk = gather_nodes // CHUNK
    local = gather_nodes % CHUNK
    calls = []
    for c in range(NCHUNK):
        m = chunk == c
        loc = local[m]
        seg = seg_nodes[m]
        # sort by seg, then round-robin rank within seg
        order = np.argsort(seg, kind="stable")
        loc, seg = loc[order], seg[order]
        # rank j of each edge within its segment
        if seg.size:
            first = np.r_[True, seg[1:] != seg[:-1]]
            idx_of_first = np.maximum.accumulate(np.where(first, np.arange(seg.size), 0))
            rank = np.arange(seg.size) - idx_of_first
        else:
            rank = np.zeros(0, np.int64)
        # groups by rank; within a rank all segs unique
        for j in range(int(rank.max()) + 1 if rank.size else 0):
            mj = rank == j
            lj, sj = loc[mj], seg[mj]
            for s in range(0, lj.size, CALL):
                calls.append((c, lj[s:s + CALL], sj[s:s + CALL]))
    return calls


def _pad_calls(calls, ncalls):
    """Pad call list to ncalls uniform calls of CALL idxs (zero-row pads)."""
    out = []
    for c, lj, sj in calls:
        g = np.full(CALL, CHUNK, np.int16)       # zero-row of the chunk
        s = np.zeros(CALL, np.int16)             # add zeros to node 0
        g[:lj.size] = lj.astype(np.int16)
        s[:sj.size] = sj.astype(np.int16)
        out.append((c, g, s))
    while len(out) < ncalls:
        out.append((0, np.full(CALL, CHUNK, np.int16), np.zeros(CALL, np.int16)))
    return out


def _wrap16(a):
    # idx i -> [i%16, i//16], replicated to 128 partitions
    return np.tile(a.reshape(-1, 16).T, (8, 1))


def kernel(x, W_self, b_self, W_s2d, b_s2d, W_d2s, b_d2s, edge_index):
    x = np.asarray(x, np.float32)
    W_self = np.asarray(W_self, np.float32)
    b_self = np.asarray(b_self, np.float32)
    W_s2d = np.asarray(W_s2d, np.float32)
    b_s2d = np.asarray(b_s2d, np.float32)
    W_d2s = np.asarray(W_d2s, np.float32)
    b_d2s = np.asarray(b_d2s, np.float32)
    src = np.asarray(edge_index[0], np.int64)
    dst = np.asarray(edge_index[1], np.int64)

    # x_aug: 4 chunks of 32768 rows; chunk c rows [32768c..] = x[32767c .. 32767c+32766], last slot zero
    x_aug = np.zeros((NCHUNK * 32768, D), np.float32)
    for c in range(NCHUNK):
        lo = c * CHUNK
        hi = min(lo + CHUNK, N)
        x_aug[c * 32768: c * 32768 + (hi - lo)] = x[lo:hi]

    x_T = np.zeros((D, P * NSH_PAD), np.float32)
    x_T[:, :N] = 0.0
    # per-core padded transposed x of own shard
    for core in range(P):
        xo = x[core * NSH:(core + 1) * NSH]
        x_T[:, core * NSH_PAD: core * NSH_PAD + NSH] = xo.T

    deg_in = np.bincount(dst, minlength=N).astype(np.float32)
    deg_out = np.bincount(src, minlength=N).astype(np.float32)
    inv_in = 1.0 / np.maximum(deg_in, 1.0)
    inv_out = 1.0 / np.maximum(deg_out, 1.0)

    W1 = (1.0 - ALPHA) * W_s2d
    W2 = ALPHA * W_d2s
    b_tot = b_self + (1.0 - ALPHA) * b_s2d + ALPHA * b_d2s

    # --- per-core plans ---
    plans = []
    for core in range(P):
        lo, hi = core * NSH, (core + 1) * NSH
        m_in = (dst >= lo) & (dst < hi)
        calls_in = _plan_direction(src[m_in], dst[m_in] - lo, core)
        m_out = (src >= lo) & (src < hi)
        calls_out = _plan_direction(dst[m_out], src[m_out] - lo, core)
        plans.append((calls_in, calls_out))
    # uniform schedule: per chunk, #calls = max over cores
    def make_sched(idx):
        counts = np.zeros((P, NCHUNK), np.int64)
        for core in range(P):
            for c, _, _ in plans[core][idx]:
                counts[core, c] += 1
        per_chunk = counts.max(axis=0)
        sched = []
        for c in range(NCHUNK):
            sched += [c] * int(per_chunk[c])
        return sched, per_chunk

    sched_in, pc_in = make_sched(0)
    sched_out, pc_out = make_sched(1)

    def conform(calls, per_chunk):
        # place each core's chunk-c calls into that chunk's slot block
        out = []
        for c in range(NCHUNK):
            mine = [t for t in calls if t[0] == c]
            for t in mine:
                _, lj, sj = t
                g = np.full(CALL, CHUNK, np.int16)
                s = np.zeros(CALL, np.int16)
                g[:lj.size] = lj.astype(np.int16)
                s[:sj.size] = sj.astype(np.int16)
                out.append((c, g, s))
            for _ in range(int(per_chunk[c]) - len(mine)):
                out.append((c, np.full(CALL, CHUNK, np.int16),
                            np.zeros(CALL, np.int16)))
        return out

    padded = []
    for core in range(P):
        padded.append((conform(plans[core][0], pc_in),
                       conform(plans[core][1], pc_out)))

    # --- build per-core input arrays ---
    def build_idx(calls):
        g = np.concatenate([_wrap16(c[1]) for c in calls], axis=1)
        s = np.concatenate([_wrap16(c[2]) for c in calls], axis=1)
        return g.astype(np.int16), s.astype(np.int16)

    in_maps = []
    for core in range(P):
        gi, si = build_idx(padded[core][0])
        go, so = build_idx(padded[core][1])
        lo = core * NSH
        invi = np.zeros(NSH_PAD, np.float32)
        invo = np.zeros(NSH_PAD, np.float32)
        invi[:NSH] = inv_in[lo:lo + NSH]
        invo[:NSH] = inv_out[lo:lo + NSH]
        in_maps.append({
            "x_aug": x_aug,
            "x_ownT": np.ascontiguousarray(
                x_T[:, core * NSH_PAD:(core + 1) * NSH_PAD]),
            "gidx_in": gi, "sidx_in": si,
            "gidx_out": go, "sidx_out": so,
            "inv_in": invi.reshape(98, 128).T.copy(),
            "inv_out": invo.reshape(98, 128).T.copy(),
            "W_self": W_self, "W1": W1, "W2": W2,
            "b": b_tot.reshape(128, 1),
            "zeros": np.zeros((NSH_PAD, D), np.float32),
            "ident": np.eye(128, dtype=np.float32),
        })

    nc = _build_program(sched_in, sched_out)
    from concourse.bass_utils import run_bass_kernel_spmd
    res = run_bass_kernel_spmd(nc, in_maps, list(range(P)))
    out = np.empty((N, D), np.float32)
    for core in range(P):
        out[core * NSH:(core + 1) * NSH] = res.results[core]["out"][:NSH]
    return out


def _build_program(sched_in, sched_out, niter=1):
    from concourse import bacc, tile, mybir, library_config
    import concourse.bass as bass

    f32 = mybir.dt.float32
    i16 = mybir.dt.int16
    nc = bacc.Bacc("TRN2", target_bir_lowering=False, debug=False,
                   num_swdge_queues=2)

    ncalls_in, ncalls_out = len(sched_in), len(sched_out)
    xa = nc.dram_tensor("x_aug", [NCHUNK * 32768, D], f32, kind="ExternalInput")
    xT = nc.dram_tensor("x_ownT", [D, NSH_PAD], f32, kind="ExternalInput")
    gii = nc.dram_tensor("gidx_in", [128, ncalls_in * CALL // 16], i16, kind="ExternalInput")
    sii = nc.dram_tensor("sidx_in", [128, ncalls_in * CALL // 16], i16, kind="ExternalInput")
    gio = nc.dram_tensor("gidx_out", [128, ncalls_out * CALL // 16], i16, kind="ExternalInput")
    sio = nc.dram_tensor("sidx_out", [128, ncalls_out * CALL // 16], i16, kind="ExternalInput")
    ivi = nc.dram_tensor("inv_in", [128, 98], f32, kind="ExternalInput")
    ivo = nc.dram_tensor("inv_out", [128, 98], f32, kind="ExternalInput")
    Ws = nc.dram_tensor("W_self", [D, D], f32, kind="ExternalInput")
    W1 = nc.dram_tensor("W1", [D, D], f32, kind="ExternalInput")
    W2 = nc.dram_tensor("W2", [D, D], f32, kind="ExternalInput")
    bt = nc.dram_tensor("b", [D, 1], f32, kind="ExternalInput")
    zz = nc.dram_tensor("zeros", [NSH_PAD, D], f32, kind="ExternalInput")
    idn = nc.dram_tensor("ident", [D, D], f32, kind="ExternalInput")
    agg_in = nc.dram_tensor("agg_in", [NSH_PAD, D], f32)
    agg_out = nc.dram_tensor("agg_out", [NSH_PAD, D], f32)
    out = nc.dram_tensor("out", [NSH_PAD, D], f32, kind="ExternalOutput")

    COLS = CALL // 16  # idx cols per call

    with tile.TileContext(nc) as tc:
        nc.gpsimd.load_library(library_config.mlp)
        with tc.tile_pool(name="const", bufs=1) as cp, \
             tc.tile_pool(name="gt", bufs=2) as gp, \
             tc.tile_pool(name="ep", bufs=3) as ep, \
             tc.tile_pool(name="ps", bufs=2, space="PSUM") as pp:
            # constants
            gii_s = cp.tile([128, ncalls_in * COLS], i16)
            sii_s = cp.tile([128, ncalls_in * COLS], i16)
            gio_s = cp.tile([128, ncalls_out * COLS], i16)
            sio_s = cp.tile([128, ncalls_out * COLS], i16)
            nc.sync.dma_start(gii_s[:], gii[:])
            nc.sync.dma_start(sii_s[:], sii[:])
            nc.sync.dma_start(gio_s[:], gio[:])
            nc.sync.dma_start(sio_s[:], sio[:])
            ivi_s = cp.tile([128, 98], f32)
            ivo_s = cp.tile([128, 98], f32)
            nc.sync.dma_start(ivi_s[:], ivi[:])
            nc.sync.dma_start(ivo_s[:], ivo[:])
            Ws_s = cp.tile([D, D], f32)
            W1_s = cp.tile([D, D], f32)
            W2_s = cp.tile([D, D], f32)
            b_s = cp.tile([D, 1], f32)
            id_s = cp.tile([D, D], f32)
            nc.sync.dma_start(Ws_s[:], Ws[:])
            nc.sync.dma_start(W1_s[:], W1[:])
            nc.sync.dma_start(W2_s[:], W2[:])
            nc.sync.dma_start(b_s[:], bt[:])
            nc.sync.dma_start(id_s[:], idn[:])

            # zero agg buffers
            nc.sync.dma_start(agg_in[:], zz[:])
            nc.sync.dma_start(agg_out[:], zz[:])

            # gather + scatter chains, both directions interleaved
            def do_dir(sched, gidx_s, sidx_s, agg):
                for k, c in enumerate(sched):
                    t = gp.tile([128, CALL_PAD // 128, D], f32, tag="gath")
                    nc.gpsimd.dma_gather(
                        t[:], xa[c * 32768:(c + 1) * 32768, :],
                        gidx_s[:, k * COLS:(k + 1) * COLS],
                        CALL, CALL, D, queue_num=0)
                    nc.gpsimd.dma_scatter_add(
                        agg[:], t[:],
                        sidx_s[:, k * COLS:(k + 1) * COLS],
                        CALL, CALL, D, queue_num=1)

            do_dir(sched_in, gii_s, sii_s, agg_in)
            do_dir(sched_out, gio_s, sio_s, agg_out)

            # epilogue per 128-node tile
            for t in range(98):
                ai = ep.tile([128, D], f32, tag="ai")
                ao = ep.tile([128, D], f32, tag="ao")
                nc.sync.dma_start(ai[:], agg_in[t * 128:(t + 1) * 128, :])
                nc.sync.dma_start(ao[:], agg_out[t * 128:(t + 1) * 128, :])
                # scale by inv degree (per-partition scalar)
                nc.vector.tensor_scalar(ai[:], ai[:], ivi_s[:, t:t + 1], None,
                                        mybir.AluOpType.mult)
                nc.vector.tensor_scalar(ao[:], ao[:], ivo_s[:, t:t + 1], None,
                                        mybir.AluOpType.mult)
                # transpose both
                pt = pp.tile([128, D], f32, tag="pt")
                nc.tensor.matmul(pt[:], ai[:], id_s[:], start=True, stop=True,
                                 is_transpose=True)
                aiT = ep.tile([128, D], f32, tag="aiT")
                nc.vector.tensor_copy(aiT[:], pt[:])
                pt2 = pp.tile([128, D], f32, tag="pt")
                nc.tensor.matmul(pt2[:], ao[:], id_s[:], start=True, stop=True,
                                 is_transpose=True)
                aoT = ep.tile([128, D], f32, tag="aoT")
                nc.vector.tensor_copy(aoT[:], pt2[:])
                # x_ownT tile direct from DRAM
                xt_t = ep.tile([128, 128], f32, tag="xt")
                nc.sync.dma_start(xt_t[:], xT[:, t * 128:(t + 1) * 128])
                # y = W_self.T @ xT + W1.T @ aiT + W2.T @ aoT   [feat_out, nodes]
                y = pp.tile([128, 128], f32, tag="y")
                nc.tensor.matmul(y[:], Ws_s[:], xt_t[:], start=True, stop=False)
                nc.tensor.matmul(y[:], W1_s[:], aiT[:], start=False, stop=False)
                nc.tensor.matmul(y[:], W2_s[:], aoT[:], start=False, stop=True)
                ysb = ep.tile([128, 128], f32, tag="ysb")
                nc.vector.tensor_scalar(ysb[:], y[:], b_s[:, 0:1], None,
                                        mybir.AluOpType.add)
                # transpose back to [nodes, feat]
                po = pp.tile([128, 128], f32, tag="po")
                nc.tensor.matmul(po[:], ysb[:], id_s[:], start=True, stop=True,
                                 is_transpose=True)
                osb = ep.tile([128, 128], f32, tag="osb")
                nc.vector.tensor_copy(osb[:], po[:])
                nc.sync.dma_start(out[t * 128:(t + 1) * 128, :], osb[:])

    nc.compile()
    return nc
